# revision 10
# baseline (speedup 1.0000x reference)
"""Fused sparse-attention kernel for Trainium2 (8 NeuronCores, data-parallel over batch).

Computation (per batch element b):
    X[s,k]  = enc[b] @ W_enc + dec_proj[b,k] + cov[b,s]*Wcovsum[k] + bias[k]
    T       = tanh(X)
    att[s]  = T @ v_w                      (+ v_b, which cancels in softmax)
    w       = softmax(att masked to s < len[b])
    new_cov = cov + w

Sharding: batch B=32 is split 4-per-core across 8 cores; all weights replicated
(per the data-parallel sharding hint).

v3 pipeline (fp8 DoubleRowSwInterleave), per batch element:
  1. SWDGE cast-DMA: enc[b] fp32 DRAM -> fp8e4m3 SBUF [s,h] tile directly
     (s on partitions; 512B descriptors). No DRAM bounce, no xbar transpose.
  2. PE transposes of the fp8 data viewed as uint16 h-PAIRS: 32x [128s,128pair]
     tiles -> 4 PSUM banks (uint16 keeps the 2-byte packed PSUM layout that
     walrus requires; fp8 transposes demand element-step 2). After transpose,
     partition hh holds s-major interleaved fp8 pairs (h=2hh, h=2hh+1) --
     exactly the contiguous A/B-interleaved stationary layout that the
     DoubleRowSwInterleave matmul mode consumes.
  3. DVE copies each bank -> SBUF et2[pc] tiles, bitcast uint32 (2048 fp8
     move as 512 elems/lane).
  4. PE per s-tile psum group: K=2 bf16 rank-1 (ones,cov~) x (16*(dec_proj+b),
     16*Wcovsum) + 2 fp8 DoubleRowSwInterleave matmuls (K=256 each, 0.5
     cyc/row) with moving W2[pc][hh,t,k] = 16*W_enc[pc*256+2hh+t, k].
     SwInterleave reads stationary columns reversed, so out partition p within
     an s-tile is s = 128j + 127 - p; all downstream per-partition constants
     (iota, cov tiles, rank-1 cov rows) and the host unshard are flipped to
     match. The x16 W scaling keeps W_enc (std 0.02) out of fp8e4m3's
     denormal range; tanh's scale=1/16 undoes it.
  5. ACT: tanh(psum * 1/16) -> bf16 T tiles.
  6. DVE: fused T*v multiply + free-dim reduce -> att column [128,1].
  7. Tiny masked softmax tail in [s_lo=128, s_hi=16] layout: exp on ACT,
     iota<len mask fused with the exp multiply on DVE, sum + 1/sum broadcast
     via two small PE matmuls (softmax max-subtraction skipped: |logits| <=
     ||v||_1 ~ 8, and v_b cancels in softmax).
"""

import numpy as np
import ml_dtypes

B, S, H, E = 32, 2048, 512, 512
NCORES = 8
BPC = B // NCORES           # batches per core
SLO, SHI = 128, S // 128    # att tile layout: s = 128*j + (127-p)  ->  [p, j]
NPC = 2                     # pair-chunks of 128 uint16 pairs (256 h) each
BF16 = ml_dtypes.bfloat16
F8E4 = ml_dtypes.float8_e4m3
WSCALE = 16.0

_CACHE = {}


def _build_nc():
    import concourse.mybir as mybir
    import concourse.tile as tile
    from concourse import bacc
    from contextlib import ExitStack

    dt = mybir.dt
    F32, BF, F8, F16 = dt.float32, dt.bfloat16, dt.float8e4, dt.float16

    nc = bacc.Bacc("TRN2", target_bir_lowering=False, debug=False,
                   enable_asserts=False, num_devices=NCORES)

    # ---- DRAM I/O (per-core shapes) ----
    enc_f32 = nc.dram_tensor("enc_f32", [BPC, S, H], F32, kind="ExternalInput").ap()
    # fp8 moving weights: per pc, per t: 512 cols of W_enc*16
    wdr = nc.dram_tensor("wdr", [128, NPC * 2 * H], F8, kind="ExternalInput").ap()
    idn = nc.dram_tensor("idn", [128, 128], F16, kind="ExternalInput").ap()
    # f32 blob: [iota (SHI) | lens (BPC) | cov_t (BPC*SHI)]   (s-flipped layout)
    fblob = nc.dram_tensor("fblob", [SLO, SHI + BPC + BPC * SHI], F32,
                           kind="ExternalInput").ap()
    r1lhs = nc.dram_tensor("r1lhs", [2, BPC * S], BF, kind="ExternalInput").ap()
    r1rhs = nc.dram_tensor("r1rhs", [2, BPC * H], BF, kind="ExternalInput").ap()
    # bf16 row consts: [vbc (H) | ones col (1)] per partition
    vbc = nc.dram_tensor("vbc", [128, H + 1], BF, kind="ExternalInput").ap()
    brow = nc.dram_tensor("brow", [1, 128], F32, kind="ExternalInput").ap()
    att_out = nc.dram_tensor("att_out", [BPC, SLO, SHI], F32, kind="ExternalOutput").ap()
    cov_out = nc.dram_tensor("cov_out", [BPC, SLO, SHI], F32, kind="ExternalOutput").ap()

    AF = mybir.ActivationFunctionType
    OP = mybir.AluOpType
    PM = mybir.MatmulPerfMode

    with tile.TileContext(nc) as tc, ExitStack() as ctx:
        consts = ctx.enter_context(tc.tile_pool(name="consts", bufs=1))
        encp = ctx.enter_context(tc.tile_pool(name="encp", bufs=2))
        etp = ctx.enter_context(tc.tile_pool(name="etp", bufs=2))
        tpool = ctx.enter_context(tc.tile_pool(name="tpool", bufs=4))
        spool = ctx.enter_context(tc.tile_pool(name="spool", bufs=2))
        small = ctx.enter_context(tc.tile_pool(name="small", bufs=2))
        attp = ctx.enter_context(tc.tile_pool(name="attp", bufs=2))
        ppt = ctx.enter_context(tc.tile_pool(name="ppt", bufs=1, space="PSUM"))
        ppm = ctx.enter_context(tc.tile_pool(name="ppm", bufs=2, space="PSUM"))
        pps = ctx.enter_context(tc.tile_pool(name="pps", bufs=1, space="PSUM"))

        # ---- one-time constant loads (emitted first on the Pool queue) ----
        wdr_sb = consts.tile([128, NPC * 2 * H], F8, tag="wdr")
        nc.gpsimd.dma_start(wdr_sb[:], wdr[:])
        idn_sb = consts.tile([128, 128], F16, tag="idn")
        nc.gpsimd.dma_start(idn_sb[:], idn[:])
        fb_sb = consts.tile([SLO, SHI + BPC + BPC * SHI], F32, tag="fblob")
        nc.gpsimd.dma_start(fb_sb[:], fblob[:])
        r1lhs_sb = consts.tile([2, BPC * S], BF, tag="r1lhs")
        nc.gpsimd.dma_start(r1lhs_sb[:], r1lhs[:])
        r1rhs_sb = consts.tile([2, BPC * H], BF, tag="r1rhs")
        nc.gpsimd.dma_start(r1rhs_sb[:], r1rhs[:])
        vbc_sb = consts.tile([128, H + 1], BF, tag="vbc")
        nc.gpsimd.dma_start(vbc_sb[:], vbc[:])
        brow_sb = consts.tile([1, 128], F32, tag="brow")
        nc.gpsimd.dma_start(brow_sb[:], brow[:])

        iota_sb = fb_sb[:, 0:SHI]
        lens_sb = fb_sb[:, SHI:SHI + BPC]
        covt_sb = fb_sb[:, SHI + BPC:]
        ones_c_sb = vbc_sb[:, H:H + 1]                     # [128,1] bf16 ones
        ones_r_sb = brow_sb                                # [1,128] f32 ones

        def wdr_ap(pc):  # [128, 2, H] fp8 moving pair weights
            return wdr_sb[:, pc * 2 * H:(pc + 1) * 2 * H].rearrange(
                "p (t k) -> p t k", t=2)

        # ---- per-batch cast load: fp32 DRAM -> fp8 SBUF [s,h], two halves ----
        def load_batch(b):
            e8 = encp.tile([128, SHI * H], F8, tag="enc8")
            src = enc_f32[b].rearrange("(j p) h -> p j h", p=128)
            dst = e8[:].rearrange("p (j h) -> p j h", h=H)
            hf = SHI // 2
            for half in range(2):
                nc.gpsimd.dma_start(
                    dst[:, half * hf:(half + 1) * hf],
                    src[:, half * hf:(half + 1) * hf])
            return e8

        pre = {0: load_batch(0)}

        # ---- main loop ----
        for b in range(BPC):
            e8 = pre.pop(b)
            e8u = e8[:].bitcast(mybir.dt.float16)    # [128, SHI*H/2] pair view

            # PE transposes: [128s, 128pair] tiles through 2 PSUM banks in 4
            # rounds of (bh, pc); gpsimd copies each bank out to SBUF et2.
            et2 = [etp.tile([128, 2 * S], F8, tag=f"et2_{pc}", name=f"et2_{pc}")
                   for pc in range(NPC)]
            for rnd, (bh, pc) in enumerate(
                    (bh, pc) for bh in range(2) for pc in range(NPC)):
                pt = ppt.tile([128, 1024], F16, tag=f"pt{rnd % 2}",
                              name=f"pt{rnd % 2}")
                for j in range(bh * 8, bh * 8 + 8):
                    nc.tensor.matmul(
                        pt[:, (j % 8) * 128:(j % 8 + 1) * 128],
                        e8u[:, j * (H // 2) + pc * 128:
                            j * (H // 2) + (pc + 1) * 128],
                        idn_sb[:],
                        start=(j % 8 == 0), stop=(j % 8 == 7),
                        is_transpose=True, skip_group_check=True,
                    )
                dst = et2[pc][:, bh * S:(bh + 1) * S]
                nc.vector.tensor_copy(dst.bitcast(mybir.dt.uint32),
                                      pt[:].bitcast(mybir.dt.uint32))

            # prefetch next batch cast while PE continues
            if b + 1 < BPC:
                pre[b + 1] = load_batch(b + 1)

            att_t = attp.tile([SLO, SHI], F32, tag="att")
            for jp in range(SHI // 2):
                ps = ppm.tile([128, 2 * H], F32, tag="x")
                for g in range(2):
                    j = 2 * jp + g
                    psg = ps[:, g * H:(g + 1) * H]
                    nc.tensor.matmul(
                        psg,
                        r1lhs_sb[:, b * S + j * 128: b * S + (j + 1) * 128],
                        r1rhs_sb[:, b * H:(b + 1) * H],
                        start=True, stop=False, skip_group_check=True,
                    )
                    for pc in range(NPC):
                        nc.tensor.matmul(
                            psg,
                            et2[pc][:, j * 256:(j + 1) * 256],
                            wdr_ap(pc),
                            start=False, stop=(pc == NPC - 1),
                            perf_mode=PM.DoubleRowSwInterleave,
                            skip_group_check=True,
                        )
                t_t = tpool.tile([128, 2 * H], BF, tag="t")
                nc.scalar.activation(t_t[:], ps[:], AF.Tanh, scale=1.0 / WSCALE)
                for g in range(2):
                    j = 2 * jp + g
                    scr = spool.tile([128, H], BF, tag="scr")
                    nc.vector.scalar_tensor_tensor(
                        out=scr[:], in0=t_t[:, g * H:(g + 1) * H], scalar=1.0,
                        in1=vbc_sb[:, 0:H],
                        op0=OP.mult, op1=OP.mult,
                        accum_out=att_t[:, j:j + 1],
                    )

            # ---- masked softmax tail (tiny) ----
            expt = small.tile([SLO, SHI], F32, tag="expt")
            nc.scalar.activation(expt[:], att_t[:], AF.Exp)
            mexp = small.tile([SLO, SHI], F32, tag="mexp")
            nc.vector.scalar_tensor_tensor(
                out=mexp[:], in0=iota_sb, scalar=lens_sb[:, b:b + 1],
                in1=expt[:], op0=OP.is_lt, op1=OP.mult,
            )
            mexp16 = small.tile([SLO, SHI], BF, tag="mexp16")
            nc.vector.tensor_copy(mexp16[:], mexp[:])
            pst = pps.tile([128, 32], F32, tag="smax")
            sum_ps = pst[0:1, 0:SHI]
            nc.tensor.matmul(sum_ps, ones_c_sb, mexp16[:],
                             start=True, stop=True, skip_group_check=True)
            ssum = small.tile([1, 1], F32, tag="ssum")
            nc.vector.reduce_sum(ssum[:], sum_ps, axis=mybir.AxisListType.X)
            sinv = small.tile([1, 1], F32, tag="sinv")
            nc.vector.reciprocal(sinv[:], ssum[:])
            inv_ps = pst[:, 16:17]
            nc.tensor.matmul(inv_ps, ones_r_sb, sinv[:], start=True, stop=True,
                             skip_group_check=True)
            wts = small.tile([SLO, SHI], F32, tag="wts")
            nc.vector.tensor_scalar(wts[:], mexp[:], inv_ps, None, OP.mult)
            nc.sync.dma_start(att_out[b], wts[:])
            ncov = small.tile([SLO, SHI], F32, tag="ncov")
            nc.vector.tensor_tensor(ncov[:], wts[:],
                                    covt_sb[:, b * SHI:(b + 1) * SHI], OP.add)
            nc.sync.dma_start(cov_out[b], ncov[:])

    nc.compile()
    return nc


def _get_nc():
    if "nc" not in _CACHE:
        _CACHE["nc"] = _build_nc()
    return _CACHE["nc"]


def _prep_in_maps(dec_input, enc_output, text_lengths, coverage_vector, W, b, v_w):
    enc = np.ascontiguousarray(np.asarray(enc_output, dtype=np.float32))
    dec = np.asarray(dec_input, dtype=np.float32).reshape(B, E)
    cov = np.asarray(coverage_vector, dtype=np.float32)
    W = np.asarray(W, dtype=np.float32)
    b = np.asarray(b, dtype=np.float32)
    v_w = np.asarray(v_w, dtype=np.float32)
    lens_f = np.asarray(text_lengths).astype(np.float32)

    wenc16 = (W[:H] * WSCALE).astype(F8E4)      # [h, k] fp8, x16
    wcovsum = W[H + E:].sum(axis=0, dtype=np.float32)
    dec_proj = dec @ W[H:H + E]                 # (B, H) fp32 on host
    vbc = np.empty((128, H + 1), BF16)
    vbc[:, :H] = v_w.astype(BF16)[None, :]
    vbc[:, H] = BF16(1.0)
    # SwInterleave reverses stationary columns: partition p <-> s = 128j+127-p
    iota = ((127.0 - np.arange(SLO, dtype=np.float32))[:, None]
            + 128.0 * np.arange(SHI, dtype=np.float32)[None, :])
    brow = np.ones((1, 128), np.float32)
    # cov in [p, j] layout with the s flip inside each 128-block
    cov_pj = cov.reshape(B, SHI, SLO)[:, :, ::-1].transpose(0, 2, 1)  # [B,128,SHI]

    wdr = np.zeros((128, NPC * 2 * H), F8E4)
    hh = np.arange(128)
    for pc in range(NPC):
        for t in range(2):
            rows = wenc16[pc * 256 + 2 * hh + t]            # [128, H]
            wdr[:, (pc * 2 + t) * H:(pc * 2 + t + 1) * H] = rows
    idn = np.eye(128, dtype=np.float16)

    in_maps = []
    for core in range(NCORES):
        sl = slice(core * BPC, (core + 1) * BPC)

        fblob = np.empty((SLO, SHI + BPC + BPC * SHI), np.float32)
        fblob[:, 0:SHI] = iota
        fblob[:, SHI:SHI + BPC] = lens_f[sl][None, :]
        fblob[:, SHI + BPC:] = cov_pj[sl].transpose(1, 0, 2).reshape(SLO, BPC * SHI)

        r1l = np.empty((2, BPC * S), BF16)
        r1l[0] = BF16(1.0)
        # r1 columns map straight to out partitions: use the flipped layout
        r1l[1] = (cov_pj[sl].astype(BF16).transpose(0, 2, 1).reshape(-1))

        r1r = np.empty((2, BPC * H), np.float32)
        r1r[0] = (WSCALE * (dec_proj[sl] + b[None, :])).reshape(-1)
        r1r[1] = np.broadcast_to(WSCALE * wcovsum, (BPC, H)).reshape(-1)

        in_maps.append({
            "enc_f32": enc[sl],
            "wdr": wdr,
            "idn": idn,
            "fblob": fblob,
            "r1lhs": r1l,
            "r1rhs": r1r.astype(BF16),
            "vbc": vbc,
            "brow": brow,
        })
    return in_maps


def kernel(dec_input, enc_output, text_lengths, coverage_vector, W, b, v_w, v_b):
    from concourse.bass_utils import run_bass_kernel_spmd

    nc = _get_nc()
    in_maps = _prep_in_maps(dec_input, enc_output, text_lengths,
                            coverage_vector, W, b, v_w)
    res = run_bass_kernel_spmd(nc, in_maps, core_ids=list(range(NCORES)))

    att = np.empty((B, S), np.float32)
    ncov = np.empty((B, S), np.float32)
    for core in range(NCORES):
        r = res.results[core]
        # undo the per-128-block s flip: out partition p is s = 128j + 127 - p
        att[core * BPC:(core + 1) * BPC] = \
            r["att_out"][:, ::-1, :].transpose(0, 2, 1).reshape(BPC, S)
        ncov[core * BPC:(core + 1) * BPC] = \
            r["cov_out"][:, ::-1, :].transpose(0, 2, 1).reshape(BPC, S)
    return att, ncov


# revision 15
# speedup vs baseline: 1.0172x; 1.0172x over previous
"""Fused sparse-attention kernel for Trainium2 (8 NeuronCores, data-parallel over batch).

Computation (per batch element b):
    X[s,k]  = enc[b] @ W_enc + dec_proj[b,k] + cov[b,s]*Wcovsum[k] + bias[k]
    T       = tanh(X)
    att[s]  = T @ v_w                      (+ v_b, which cancels in softmax)
    w       = softmax(att masked to s < len[b])
    new_cov = cov + w

Sharding: batch B=32 is split 4-per-core across 8 cores; all weights replicated
(per the data-parallel sharding hint).

v3 pipeline (fp8 DoubleRowSwInterleave), per batch element:
  1. SWDGE cast-DMA: enc[b] fp32 DRAM -> fp8e4m3 SBUF [s,h] tile directly
     (s on partitions; 512B descriptors). No DRAM bounce, no xbar transpose.
  2. PE transposes of the fp8 data viewed as uint16 h-PAIRS: 32x [128s,128pair]
     tiles -> 4 PSUM banks (uint16 keeps the 2-byte packed PSUM layout that
     walrus requires; fp8 transposes demand element-step 2). After transpose,
     partition hh holds s-major interleaved fp8 pairs (h=2hh, h=2hh+1) --
     exactly the contiguous A/B-interleaved stationary layout that the
     DoubleRowSwInterleave matmul mode consumes.
  3. DVE copies each bank -> SBUF et2[pc] tiles, bitcast uint32 (2048 fp8
     move as 512 elems/lane).
  4. PE per s-tile psum group: K=2 bf16 rank-1 (ones,cov~) x (16*(dec_proj+b),
     16*Wcovsum) + 2 fp8 DoubleRowSwInterleave matmuls (K=256 each, 0.5
     cyc/row) with moving W2[pc][hh,t,k] = 16*W_enc[pc*256+2hh+t, k].
     SwInterleave reads stationary columns reversed, so out partition p within
     an s-tile is s = 128j + 127 - p; all downstream per-partition constants
     (iota, cov tiles, rank-1 cov rows) and the host unshard are flipped to
     match. The x16 W scaling keeps W_enc (std 0.02) out of fp8e4m3's
     denormal range; tanh's scale=1/16 undoes it.
  5. ACT: tanh(psum * 1/16) -> bf16 T tiles.
  6. DVE: fused T*v multiply + free-dim reduce -> att column [128,1].
  7. Tiny masked softmax tail in [s_lo=128, s_hi=16] layout: exp on ACT,
     iota<len mask fused with the exp multiply on DVE, sum + 1/sum broadcast
     via two small PE matmuls (softmax max-subtraction skipped: |logits| <=
     ||v||_1 ~ 8, and v_b cancels in softmax).
"""

import numpy as np
import ml_dtypes

B, S, H, E = 32, 2048, 512, 512
NCORES = 8
BPC = B // NCORES           # batches per core
SLO, SHI = 128, S // 128    # att tile layout: s = 128*j + (127-p)  ->  [p, j]
NPC = 2                     # pair-chunks of 128 uint16 pairs (256 h) each
BF16 = ml_dtypes.bfloat16
F8E4 = ml_dtypes.float8_e4m3
WSCALE = 16.0

_CACHE = {}


def _build_nc():
    import concourse.mybir as mybir
    import concourse.tile as tile
    from concourse import bacc
    from contextlib import ExitStack

    dt = mybir.dt
    F32, BF, F8, F16 = dt.float32, dt.bfloat16, dt.float8e4, dt.float16

    nc = bacc.Bacc("TRN2", target_bir_lowering=False, debug=False,
                   enable_asserts=False, num_devices=NCORES)

    # ---- DRAM I/O (per-core shapes) ----
    enc_f32 = nc.dram_tensor("enc_f32", [BPC, S, H], F32, kind="ExternalInput").ap()
    # fp8 moving weights: per pc, per t: 512 cols of W_enc*16
    wdr = nc.dram_tensor("wdr", [128, NPC * 2 * H], F8, kind="ExternalInput").ap()
    idn = nc.dram_tensor("idn", [128, 128], F16, kind="ExternalInput").ap()
    # f32 blob: [iota (SHI) | lens (BPC) | cov_t (BPC*SHI)]   (s-flipped layout)
    fblob = nc.dram_tensor("fblob", [SLO, SHI + BPC + BPC * SHI], F32,
                           kind="ExternalInput").ap()
    r1lhs = nc.dram_tensor("r1lhs", [2, BPC * S], BF, kind="ExternalInput").ap()
    r1rhs = nc.dram_tensor("r1rhs", [2, BPC * H], BF, kind="ExternalInput").ap()
    # bf16 row consts: [vbc (H) | ones col (1)] per partition
    vbc = nc.dram_tensor("vbc", [128, H + 1], BF, kind="ExternalInput").ap()
    brow = nc.dram_tensor("brow", [1, 128], F32, kind="ExternalInput").ap()
    att_out = nc.dram_tensor("att_out", [BPC, SLO, SHI], F32, kind="ExternalOutput").ap()
    cov_out = nc.dram_tensor("cov_out", [BPC, SLO, SHI], F32, kind="ExternalOutput").ap()

    AF = mybir.ActivationFunctionType
    OP = mybir.AluOpType
    PM = mybir.MatmulPerfMode

    with tile.TileContext(nc) as tc, ExitStack() as ctx:
        consts = ctx.enter_context(tc.tile_pool(name="consts", bufs=1))
        encp = ctx.enter_context(tc.tile_pool(name="encp", bufs=2))
        etp = ctx.enter_context(tc.tile_pool(name="etp", bufs=2))
        tpool = ctx.enter_context(tc.tile_pool(name="tpool", bufs=4))
        spool = ctx.enter_context(tc.tile_pool(name="spool", bufs=2))
        small = ctx.enter_context(tc.tile_pool(name="small", bufs=2))
        attp = ctx.enter_context(tc.tile_pool(name="attp", bufs=1))
        ppt = ctx.enter_context(tc.tile_pool(name="ppt", bufs=1, space="PSUM"))
        ppm = ctx.enter_context(tc.tile_pool(name="ppm", bufs=2, space="PSUM"))
        pps = ctx.enter_context(tc.tile_pool(name="pps", bufs=2, space="PSUM"))

        # ---- one-time constant loads (emitted first on the Pool queue) ----
        wdr_sb = consts.tile([128, NPC * 2 * H], F8, tag="wdr")
        nc.gpsimd.dma_start(wdr_sb[:], wdr[:])
        idn_sb = consts.tile([128, 128], F16, tag="idn")
        nc.gpsimd.dma_start(idn_sb[:], idn[:])
        fb_sb = consts.tile([SLO, SHI + BPC + BPC * SHI], F32, tag="fblob")
        nc.gpsimd.dma_start(fb_sb[:], fblob[:])
        r1lhs_sb = consts.tile([2, BPC * S], BF, tag="r1lhs")
        nc.gpsimd.dma_start(r1lhs_sb[:], r1lhs[:])
        r1rhs_sb = consts.tile([2, BPC * H], BF, tag="r1rhs")
        nc.gpsimd.dma_start(r1rhs_sb[:], r1rhs[:])
        vbc_sb = consts.tile([128, H + 1], BF, tag="vbc")
        nc.gpsimd.dma_start(vbc_sb[:], vbc[:])
        brow_sb = consts.tile([1, 128], F32, tag="brow")
        nc.gpsimd.dma_start(brow_sb[:], brow[:])

        iota_sb = fb_sb[:, 0:SHI]
        lens_sb = fb_sb[:, SHI:SHI + BPC]
        covt_sb = fb_sb[:, SHI + BPC:]
        ones_c_sb = vbc_sb[:, H:H + 1]                     # [128,1] bf16 ones
        ones_r_sb = brow_sb                                # [1,128] f32 ones

        def wdr_ap(pc):  # [128, 2, H] fp8 moving pair weights
            return wdr_sb[:, pc * 2 * H:(pc + 1) * 2 * H].rearrange(
                "p (t k) -> p t k", t=2)

        # ---- per-batch cast load: fp32 DRAM -> fp8 SBUF [s,h], two halves ----
        def load_batch(b):
            e8 = encp.tile([128, SHI * H], F8, tag="enc8")
            src = enc_f32[b].rearrange("(j p) h -> p j h", p=128)
            dst = e8[:].rearrange("p (j h) -> p j h", h=H)
            hf = SHI // 2
            for half in range(2):
                nc.gpsimd.dma_start(
                    dst[:, half * hf:(half + 1) * hf],
                    src[:, half * hf:(half + 1) * hf])
            return e8

        # PE transposes: [128s, 128pair] tiles through 2 PSUM banks in 4
        # rounds of (bh, pc); DVE copies each bank out to SBUF et2.
        def transcopy(e8):
            e8u = e8[:].bitcast(mybir.dt.float16)    # [128, SHI*H/2] pair view
            et2 = [etp.tile([128, 2 * S], F8, tag=f"et2_{pc}", name=f"et2_{pc}")
                   for pc in range(NPC)]
            for rnd, (bh, pc) in enumerate(
                    (bh, pc) for bh in range(2) for pc in range(NPC)):
                pt = ppt.tile([128, 1024], F16, tag=f"pt{rnd % 2}",
                              name=f"pt{rnd % 2}")
                for j in range(bh * 8, bh * 8 + 8):
                    nc.tensor.matmul(
                        pt[:, (j % 8) * 128:(j % 8 + 1) * 128],
                        e8u[:, j * (H // 2) + pc * 128:
                            j * (H // 2) + (pc + 1) * 128],
                        idn_sb[:],
                        start=(j % 8 == 0), stop=(j % 8 == 7),
                        is_transpose=True, skip_group_check=True,
                    )
                dst = et2[pc][:, bh * S:(bh + 1) * S]
                nc.scalar.copy(dst.bitcast(BF), pt[:].bitcast(BF))
            return et2

        pre = {0: load_batch(0)}
        et2_cur = transcopy(pre.pop(0))
        att_ts = []

        # ---- main loop (softmax tails deferred to a pipelined final phase) ----
        for b in range(BPC):
            if b + 1 < BPC:
                pre[b + 1] = load_batch(b + 1)
            et2 = et2_cur

            att_t = attp.tile([SLO, SHI], F32, tag=f"att{b}", name=f"att{b}")
            att_ts.append(att_t)
            for jp in range(SHI // 2):
                ps = ppm.tile([128, 2 * H], F32, tag="x")
                for g in range(2):
                    j = 2 * jp + g
                    psg = ps[:, g * H:(g + 1) * H]
                    nc.tensor.matmul(
                        psg,
                        r1lhs_sb[:, b * S + j * 128: b * S + (j + 1) * 128],
                        r1rhs_sb[:, b * H:(b + 1) * H],
                        start=True, stop=False, skip_group_check=True,
                    )
                    for pc in range(NPC):
                        nc.tensor.matmul(
                            psg,
                            et2[pc][:, j * 256:(j + 1) * 256],
                            wdr_ap(pc),
                            start=False, stop=(pc == NPC - 1),
                            perf_mode=PM.DoubleRowSwInterleave,
                            skip_group_check=True,
                        )
                t_t = tpool.tile([128, 2 * H], BF, tag="t")
                nc.scalar.activation(t_t[:], ps[:], AF.Tanh, scale=1.0 / WSCALE)
                for g in range(2):
                    j = 2 * jp + g
                    scr = spool.tile([128, H], BF, tag=f"scr{g}")
                    nc.vector.scalar_tensor_tensor(
                        out=scr[:], in0=t_t[:, g * H:(g + 1) * H], scalar=1.0,
                        in1=vbc_sb[:, 0:H],
                        op0=OP.mult, op1=OP.mult,
                        accum_out=att_t[:, j:j + 1],
                    )

            if b + 1 < BPC:
                et2_cur = transcopy(pre.pop(b + 1))

        # ---- masked softmax tails, pipelined across the 4 batches ----
        for b in range(BPC):
            att_t = att_ts[b]
            expt = small.tile([SLO, SHI], F32, tag="expt")
            nc.scalar.activation(expt[:], att_t[:], AF.Exp)
            mexp = small.tile([SLO, SHI], F32, tag="mexp")
            nc.vector.scalar_tensor_tensor(
                out=mexp[:], in0=iota_sb, scalar=lens_sb[:, b:b + 1],
                in1=expt[:], op0=OP.is_lt, op1=OP.mult,
            )
            mexp16 = small.tile([SLO, SHI], BF, tag="mexp16")
            nc.vector.tensor_copy(mexp16[:], mexp[:])
            pst = pps.tile([128, 32], F32, tag="smax")
            sum_ps = pst[0:1, 0:SHI]
            nc.tensor.matmul(sum_ps, ones_c_sb, mexp16[:],
                             start=True, stop=True, skip_group_check=True)
            ssum = small.tile([1, 1], F32, tag="ssum")
            nc.vector.reduce_sum(ssum[:], sum_ps, axis=mybir.AxisListType.X)
            sinv = small.tile([1, 1], F32, tag="sinv")
            nc.vector.reciprocal(sinv[:], ssum[:])
            inv_ps = pst[:, 16:17]
            nc.tensor.matmul(inv_ps, ones_r_sb, sinv[:], start=True, stop=True,
                             skip_group_check=True)
            wts = small.tile([SLO, SHI], F32, tag="wts")
            nc.vector.tensor_scalar(wts[:], mexp[:], inv_ps, None, OP.mult)
            nc.sync.dma_start(att_out[b], wts[:])
            ncov = small.tile([SLO, SHI], F32, tag="ncov")
            nc.vector.tensor_tensor(ncov[:], wts[:],
                                    covt_sb[:, b * SHI:(b + 1) * SHI], OP.add)
            nc.sync.dma_start(cov_out[b], ncov[:])

    nc.compile()
    return nc


def _get_nc():
    if "nc" not in _CACHE:
        _CACHE["nc"] = _build_nc()
    return _CACHE["nc"]


def _prep_in_maps(dec_input, enc_output, text_lengths, coverage_vector, W, b, v_w):
    enc = np.ascontiguousarray(np.asarray(enc_output, dtype=np.float32))
    dec = np.asarray(dec_input, dtype=np.float32).reshape(B, E)
    cov = np.asarray(coverage_vector, dtype=np.float32)
    W = np.asarray(W, dtype=np.float32)
    b = np.asarray(b, dtype=np.float32)
    v_w = np.asarray(v_w, dtype=np.float32)
    lens_f = np.asarray(text_lengths).astype(np.float32)

    wenc16 = (W[:H] * WSCALE).astype(F8E4)      # [h, k] fp8, x16
    wcovsum = W[H + E:].sum(axis=0, dtype=np.float32)
    dec_proj = dec @ W[H:H + E]                 # (B, H) fp32 on host
    vbc = np.empty((128, H + 1), BF16)
    vbc[:, :H] = v_w.astype(BF16)[None, :]
    vbc[:, H] = BF16(1.0)
    # SwInterleave reverses stationary columns: partition p <-> s = 128j+127-p
    iota = ((127.0 - np.arange(SLO, dtype=np.float32))[:, None]
            + 128.0 * np.arange(SHI, dtype=np.float32)[None, :])
    brow = np.ones((1, 128), np.float32)
    # cov in [p, j] layout with the s flip inside each 128-block
    cov_pj = cov.reshape(B, SHI, SLO)[:, :, ::-1].transpose(0, 2, 1)  # [B,128,SHI]

    wdr = np.zeros((128, NPC * 2 * H), F8E4)
    hh = np.arange(128)
    for pc in range(NPC):
        for t in range(2):
            rows = wenc16[pc * 256 + 2 * hh + t]            # [128, H]
            wdr[:, (pc * 2 + t) * H:(pc * 2 + t + 1) * H] = rows
    idn = np.eye(128, dtype=np.float16)

    in_maps = []
    for core in range(NCORES):
        sl = slice(core * BPC, (core + 1) * BPC)

        fblob = np.empty((SLO, SHI + BPC + BPC * SHI), np.float32)
        fblob[:, 0:SHI] = iota
        fblob[:, SHI:SHI + BPC] = lens_f[sl][None, :]
        fblob[:, SHI + BPC:] = cov_pj[sl].transpose(1, 0, 2).reshape(SLO, BPC * SHI)

        r1l = np.empty((2, BPC * S), BF16)
        r1l[0] = BF16(1.0)
        # r1 columns map straight to out partitions: use the flipped layout
        r1l[1] = (cov_pj[sl].astype(BF16).transpose(0, 2, 1).reshape(-1))

        r1r = np.empty((2, BPC * H), np.float32)
        r1r[0] = (WSCALE * (dec_proj[sl] + b[None, :])).reshape(-1)
        r1r[1] = np.broadcast_to(WSCALE * wcovsum, (BPC, H)).reshape(-1)

        in_maps.append({
            "enc_f32": enc[sl],
            "wdr": wdr,
            "idn": idn,
            "fblob": fblob,
            "r1lhs": r1l,
            "r1rhs": r1r.astype(BF16),
            "vbc": vbc,
            "brow": brow,
        })
    return in_maps


def kernel(dec_input, enc_output, text_lengths, coverage_vector, W, b, v_w, v_b):
    from concourse.bass_utils import run_bass_kernel_spmd

    nc = _get_nc()
    in_maps = _prep_in_maps(dec_input, enc_output, text_lengths,
                            coverage_vector, W, b, v_w)
    res = run_bass_kernel_spmd(nc, in_maps, core_ids=list(range(NCORES)))

    att = np.empty((B, S), np.float32)
    ncov = np.empty((B, S), np.float32)
    for core in range(NCORES):
        r = res.results[core]
        # undo the per-128-block s flip: out partition p is s = 128j + 127 - p
        att[core * BPC:(core + 1) * BPC] = \
            r["att_out"][:, ::-1, :].transpose(0, 2, 1).reshape(BPC, S)
        ncov[core * BPC:(core + 1) * BPC] = \
            r["cov_out"][:, ::-1, :].transpose(0, 2, 1).reshape(BPC, S)
    return att, ncov


# revision 18
# speedup vs baseline: 1.1974x; 1.1772x over previous
"""Fused sparse-attention kernel for Trainium2 (8 NeuronCores, data-parallel over batch).

Computation (per batch element b):
    X[s,k]  = enc[b] @ W_enc + dec_proj[b,k] + cov[b,s]*Wcovsum[k] + bias[k]
    T       = tanh(X)
    att[s]  = T @ v_w                      (+ v_b, which cancels in softmax)
    w       = softmax(att masked to s < len[b])
    new_cov = cov + w

Sharding: batch B=32 is split 4-per-core across 8 cores; all weights replicated
(per the data-parallel sharding hint).

v3 pipeline (fp8 DoubleRowSwInterleave), per batch element:
  1. SWDGE cast-DMA: enc[b] fp32 DRAM -> fp8e4m3 SBUF [s,h] tile directly
     (s on partitions; 512B descriptors). No DRAM bounce, no xbar transpose.
  2. PE transposes of the fp8 data viewed as uint16 h-PAIRS: 32x [128s,128pair]
     tiles -> 4 PSUM banks (uint16 keeps the 2-byte packed PSUM layout that
     walrus requires; fp8 transposes demand element-step 2). After transpose,
     partition hh holds s-major interleaved fp8 pairs (h=2hh, h=2hh+1) --
     exactly the contiguous A/B-interleaved stationary layout that the
     DoubleRowSwInterleave matmul mode consumes.
  3. DVE copies each bank -> SBUF et2[pc] tiles, bitcast uint32 (2048 fp8
     move as 512 elems/lane).
  4. PE per s-tile psum group: K=2 bf16 rank-1 (ones,cov~) x (16*(dec_proj+b),
     16*Wcovsum) + 2 fp8 DoubleRowSwInterleave matmuls (K=256 each, 0.5
     cyc/row) with moving W2[pc][hh,t,k] = 16*W_enc[pc*256+2hh+t, k].
     SwInterleave reads stationary columns reversed, so out partition p within
     an s-tile is s = 128j + 127 - p; all downstream per-partition constants
     (iota, cov tiles, rank-1 cov rows) and the host unshard are flipped to
     match. The x16 W scaling keeps W_enc (std 0.02) out of fp8e4m3's
     denormal range; tanh's scale=1/16 undoes it.
  5. ACT: tanh(psum * 1/16) -> bf16 T tiles.
  6. DVE: fused T*v multiply + free-dim reduce -> att column [128,1].
  7. Tiny masked softmax tail in [s_lo=128, s_hi=16] layout: exp on ACT,
     iota<len mask fused with the exp multiply on DVE, sum + 1/sum broadcast
     via two small PE matmuls (softmax max-subtraction skipped: |logits| <=
     ||v||_1 ~ 8, and v_b cancels in softmax).
"""

import numpy as np
import ml_dtypes

B, S, H, E = 32, 2048, 512, 512
NCORES = 8
BPC = B // NCORES           # batches per core
SLO, SHI = 128, S // 128    # att tile layout: s = 128*j + (127-p)  ->  [p, j]
NPC = 2                     # pair-chunks of 128 uint16 pairs (256 h) each
BF16 = ml_dtypes.bfloat16
F8E4 = ml_dtypes.float8_e4m3
WSCALE = 16.0

_CACHE = {}


def _build_nc():
    import concourse.mybir as mybir
    import concourse.tile as tile
    from concourse import bacc
    from contextlib import ExitStack

    dt = mybir.dt
    F32, BF, F8, F16 = dt.float32, dt.bfloat16, dt.float8e4, dt.float16

    nc = bacc.Bacc("TRN2", target_bir_lowering=False, debug=False,
                   enable_asserts=False, num_devices=NCORES)

    # ---- DRAM I/O (per-core shapes) ----
    enc_f32 = nc.dram_tensor("enc_f32", [BPC, S, H], F32, kind="ExternalInput").ap()
    # fp8 moving weights: per pc, per t: 512 cols of W_enc*16
    wdr = nc.dram_tensor("wdr", [128, NPC * 2 * H], F8, kind="ExternalInput").ap()
    idn = nc.dram_tensor("idn", [128, 128], F16, kind="ExternalInput").ap()
    # f32 blob: [iota (SHI) | lens (BPC) | cov_t (BPC*SHI)]   (s-flipped layout)
    fblob = nc.dram_tensor("fblob", [SLO, SHI + BPC + BPC * SHI], F32,
                           kind="ExternalInput").ap()
    r1lhs = nc.dram_tensor("r1lhs", [2, BPC * S], BF, kind="ExternalInput").ap()
    r1rhs = nc.dram_tensor("r1rhs", [2, BPC * H], BF, kind="ExternalInput").ap()
    # bf16 row consts: [vbc (H) | ones col (1)] per partition
    vbc = nc.dram_tensor("vbc", [128, H + 1], BF, kind="ExternalInput").ap()
    brow = nc.dram_tensor("brow", [1, 128], F32, kind="ExternalInput").ap()
    att_out = nc.dram_tensor("att_out", [BPC, SLO, SHI], F32, kind="ExternalOutput").ap()
    cov_out = nc.dram_tensor("cov_out", [BPC, SLO, SHI], F32, kind="ExternalOutput").ap()

    AF = mybir.ActivationFunctionType
    OP = mybir.AluOpType
    PM = mybir.MatmulPerfMode

    with tile.TileContext(nc) as tc, ExitStack() as ctx:
        consts = ctx.enter_context(tc.tile_pool(name="consts", bufs=1))
        encp = ctx.enter_context(tc.tile_pool(name="encp", bufs=2))
        etp = ctx.enter_context(tc.tile_pool(name="etp", bufs=2))
        tpool = ctx.enter_context(tc.tile_pool(name="tpool", bufs=4))
        spool = ctx.enter_context(tc.tile_pool(name="spool", bufs=2))
        small = ctx.enter_context(tc.tile_pool(name="small", bufs=2))
        attp = ctx.enter_context(tc.tile_pool(name="attp", bufs=1))
        ppt = ctx.enter_context(tc.tile_pool(name="ppt", bufs=1, space="PSUM"))
        ppm = ctx.enter_context(tc.tile_pool(name="ppm", bufs=2, space="PSUM"))
        pps = ctx.enter_context(tc.tile_pool(name="pps", bufs=2, space="PSUM"))

        # ---- one-time constant loads (emitted first on the Pool queue) ----
        wdr_sb = consts.tile([128, NPC * 2 * H], F8, tag="wdr")
        nc.sync.dma_start(wdr_sb[:], wdr[:])
        idn_sb = consts.tile([128, 128], F16, tag="idn")
        nc.sync.dma_start(idn_sb[:], idn[:])
        fb_sb = consts.tile([SLO, SHI + BPC + BPC * SHI], F32, tag="fblob")
        nc.sync.dma_start(fb_sb[:], fblob[:])
        r1lhs_sb = consts.tile([2, BPC * S], BF, tag="r1lhs")
        nc.sync.dma_start(r1lhs_sb[:], r1lhs[:])
        r1rhs_sb = consts.tile([2, BPC * H], BF, tag="r1rhs")
        nc.sync.dma_start(r1rhs_sb[:], r1rhs[:])
        vbc_sb = consts.tile([128, H + 1], BF, tag="vbc")
        nc.sync.dma_start(vbc_sb[:], vbc[:])
        brow_sb = consts.tile([1, 128], F32, tag="brow")
        nc.sync.dma_start(brow_sb[:], brow[:])

        iota_sb = fb_sb[:, 0:SHI]
        lens_sb = fb_sb[:, SHI:SHI + BPC]
        covt_sb = fb_sb[:, SHI + BPC:]
        ones_c_sb = vbc_sb[:, H:H + 1]                     # [128,1] bf16 ones
        ones_r_sb = brow_sb                                # [1,128] f32 ones

        def wdr_ap(pc):  # [128, 2, H] fp8 moving pair weights
            return wdr_sb[:, pc * 2 * H:(pc + 1) * 2 * H].rearrange(
                "p (t k) -> p t k", t=2)

        # ---- per-batch cast load: fp32 DRAM -> fp8 SBUF [s,h], two halves ----
        def load_batch(b):
            e8 = encp.tile([128, SHI * H], F8, tag="enc8")
            src = enc_f32[b].rearrange("(j p) h -> p j h", p=128)
            dst = e8[:].rearrange("p (j h) -> p j h", h=H)
            qf = SHI // 4
            for q in range(4):
                nc.gpsimd.dma_start(
                    dst[:, q * qf:(q + 1) * qf],
                    src[:, q * qf:(q + 1) * qf])
            return e8

        # PE transposes: [128s, 128pair] tiles through 2 PSUM banks in 4
        # rounds of (bh, pc); DVE copies each bank out to SBUF et2.
        def transcopy(e8, b):
            e8u = e8[:].bitcast(mybir.dt.float16)    # [128, SHI*H/2] pair view
            et2 = [etp.tile([128, 2 * S], F8, tag=f"et2_{pc}", name=f"et2_{pc}")
                   for pc in range(NPC)]
            for rnd, (bh, pc) in enumerate(
                    (bh, pc) for bh in range(2) for pc in range(NPC)):
                pt = ppt.tile([128, 1024], F16, tag=f"pt{rnd % 2}",
                              name=f"pt{rnd % 2}")
                for j in range(bh * 8, bh * 8 + 8):
                    nc.tensor.matmul(
                        pt[:, (j % 8) * 128:(j % 8 + 1) * 128],
                        e8u[:, j * (H // 2) + pc * 128:
                            j * (H // 2) + (pc + 1) * 128],
                        idn_sb[:],
                        start=(j % 8 == 0), stop=(j % 8 == 7),
                        is_transpose=True, skip_group_check=True,
                    )
                dst = et2[pc][:, bh * S:(bh + 1) * S]
                if (b * 4 + rnd) % 8 < 5:
                    nc.scalar.copy(dst.bitcast(BF), pt[:].bitcast(BF))
                else:
                    nc.vector.tensor_copy(dst.bitcast(mybir.dt.uint32),
                                          pt[:].bitcast(mybir.dt.uint32))
            return et2

        # ---- masked softmax tail, emitted in 4 chunks so the serial chain
        # interleaves with the next batch's main loop on the in-order queues ----
        tail_state = {}

        def tail_chunk(b, att_t, phase):
            st = tail_state.setdefault(b, {})
            if phase == 0:
                st["expt"] = small.tile([SLO, SHI], F32, tag="expt", name="expt")
                nc.scalar.activation(st["expt"][:], att_t[:], AF.Exp)
                st["mexp"] = small.tile([SLO, SHI], F32, tag="mexp", name="mexp")
                nc.vector.scalar_tensor_tensor(
                    out=st["mexp"][:], in0=iota_sb, scalar=lens_sb[:, b:b + 1],
                    in1=st["expt"][:], op0=OP.is_lt, op1=OP.mult,
                )
                mexp16 = small.tile([SLO, SHI], BF, tag="mexp16")
                nc.vector.tensor_copy(mexp16[:], st["mexp"][:])
                st["pst"] = pps.tile([128, 32], F32, tag="smax", name="smax")
                nc.tensor.matmul(st["pst"][0:1, 0:SHI], ones_c_sb, mexp16[:],
                                 start=True, stop=True, skip_group_check=True)
            elif phase == 1:
                ssum = small.tile([1, 1], F32, tag="ssum")
                nc.vector.reduce_sum(ssum[:], st["pst"][0:1, 0:SHI],
                                     axis=mybir.AxisListType.X)
                sinv = small.tile([1, 1], F32, tag="sinv")
                nc.vector.reciprocal(sinv[:], ssum[:])
                nc.tensor.matmul(st["pst"][:, 16:17], ones_r_sb, sinv[:],
                                 start=True, stop=True, skip_group_check=True)
            elif phase == 2:
                st["wts"] = small.tile([SLO, SHI], F32, tag="wts", name="wts")
                nc.vector.tensor_scalar(st["wts"][:], st["mexp"][:],
                                        st["pst"][:, 16:17], None, OP.mult)
                nc.sync.dma_start(att_out[b], st["wts"][:])
            else:
                ncov = small.tile([SLO, SHI], F32, tag="ncov")
                nc.vector.tensor_tensor(ncov[:], st["wts"][:],
                                        covt_sb[:, b * SHI:(b + 1) * SHI], OP.add)
                nc.sync.dma_start(cov_out[b], ncov[:])
                del tail_state[b]

        pre = {0: load_batch(0)}
        et2_cur = transcopy(pre.pop(0), 0)
        att_prev = None

        # ---- main loop ----
        for b in range(BPC):
            if b + 1 < BPC:
                pre[b + 1] = load_batch(b + 1)
            et2 = et2_cur

            att_t = attp.tile([SLO, SHI], F32, tag=f"att{b}", name=f"att{b}")
            for jp in range(SHI // 2):
                ps = ppm.tile([128, 2 * H], F32, tag="x")
                for g in range(2):
                    j = 2 * jp + g
                    psg = ps[:, g * H:(g + 1) * H]
                    nc.tensor.matmul(
                        psg,
                        r1lhs_sb[:, b * S + j * 128: b * S + (j + 1) * 128],
                        r1rhs_sb[:, b * H:(b + 1) * H],
                        start=True, stop=False, skip_group_check=True,
                    )
                    for pc in range(NPC):
                        nc.tensor.matmul(
                            psg,
                            et2[pc][:, j * 256:(j + 1) * 256],
                            wdr_ap(pc),
                            start=False, stop=(pc == NPC - 1),
                            perf_mode=PM.DoubleRowSwInterleave,
                            skip_group_check=True,
                        )
                t_t = tpool.tile([128, 2 * H], BF, tag="t")
                nc.scalar.activation(t_t[:], ps[:], AF.Tanh, scale=1.0 / WSCALE)
                for g in range(2):
                    j = 2 * jp + g
                    scr = spool.tile([128, H], BF, tag=f"scr{g}")
                    nc.vector.scalar_tensor_tensor(
                        out=scr[:], in0=t_t[:, g * H:(g + 1) * H], scalar=1.0,
                        in1=vbc_sb[:, 0:H],
                        op0=OP.mult, op1=OP.mult,
                        accum_out=att_t[:, j:j + 1],
                    )
                # previous batch's softmax tail, spread across this batch
                if att_prev is not None and jp in (1, 3, 5, 7):
                    tail_chunk(b - 1, att_prev, (jp - 1) // 2)

            if b + 1 < BPC:
                et2_cur = transcopy(pre.pop(b + 1), b + 1)
            att_prev = att_t

        # last batch's tail
        for phase in range(4):
            tail_chunk(BPC - 1, att_prev, phase)

    nc.compile()
    return nc


def _get_nc():
    if "nc" not in _CACHE:
        _CACHE["nc"] = _build_nc()
    return _CACHE["nc"]


def _prep_in_maps(dec_input, enc_output, text_lengths, coverage_vector, W, b, v_w):
    enc = np.ascontiguousarray(np.asarray(enc_output, dtype=np.float32))
    dec = np.asarray(dec_input, dtype=np.float32).reshape(B, E)
    cov = np.asarray(coverage_vector, dtype=np.float32)
    W = np.asarray(W, dtype=np.float32)
    b = np.asarray(b, dtype=np.float32)
    v_w = np.asarray(v_w, dtype=np.float32)
    lens_f = np.asarray(text_lengths).astype(np.float32)

    wenc16 = (W[:H] * WSCALE).astype(F8E4)      # [h, k] fp8, x16
    wcovsum = W[H + E:].sum(axis=0, dtype=np.float32)
    dec_proj = dec @ W[H:H + E]                 # (B, H) fp32 on host
    vbc = np.empty((128, H + 1), BF16)
    vbc[:, :H] = v_w.astype(BF16)[None, :]
    vbc[:, H] = BF16(1.0)
    # SwInterleave reverses stationary columns: partition p <-> s = 128j+127-p
    iota = ((127.0 - np.arange(SLO, dtype=np.float32))[:, None]
            + 128.0 * np.arange(SHI, dtype=np.float32)[None, :])
    brow = np.ones((1, 128), np.float32)
    # cov in [p, j] layout with the s flip inside each 128-block
    cov_pj = cov.reshape(B, SHI, SLO)[:, :, ::-1].transpose(0, 2, 1)  # [B,128,SHI]

    wdr = np.zeros((128, NPC * 2 * H), F8E4)
    hh = np.arange(128)
    for pc in range(NPC):
        for t in range(2):
            rows = wenc16[pc * 256 + 2 * hh + t]            # [128, H]
            wdr[:, (pc * 2 + t) * H:(pc * 2 + t + 1) * H] = rows
    idn = np.eye(128, dtype=np.float16)

    in_maps = []
    for core in range(NCORES):
        sl = slice(core * BPC, (core + 1) * BPC)

        fblob = np.empty((SLO, SHI + BPC + BPC * SHI), np.float32)
        fblob[:, 0:SHI] = iota
        fblob[:, SHI:SHI + BPC] = lens_f[sl][None, :]
        fblob[:, SHI + BPC:] = cov_pj[sl].transpose(1, 0, 2).reshape(SLO, BPC * SHI)

        r1l = np.empty((2, BPC * S), BF16)
        r1l[0] = BF16(1.0)
        # r1 columns map straight to out partitions: use the flipped layout
        r1l[1] = (cov_pj[sl].astype(BF16).transpose(0, 2, 1).reshape(-1))

        r1r = np.empty((2, BPC * H), np.float32)
        r1r[0] = (WSCALE * (dec_proj[sl] + b[None, :])).reshape(-1)
        r1r[1] = np.broadcast_to(WSCALE * wcovsum, (BPC, H)).reshape(-1)

        in_maps.append({
            "enc_f32": enc[sl],
            "wdr": wdr,
            "idn": idn,
            "fblob": fblob,
            "r1lhs": r1l,
            "r1rhs": r1r.astype(BF16),
            "vbc": vbc,
            "brow": brow,
        })
    return in_maps


def kernel(dec_input, enc_output, text_lengths, coverage_vector, W, b, v_w, v_b):
    from concourse.bass_utils import run_bass_kernel_spmd

    nc = _get_nc()
    in_maps = _prep_in_maps(dec_input, enc_output, text_lengths,
                            coverage_vector, W, b, v_w)
    res = run_bass_kernel_spmd(nc, in_maps, core_ids=list(range(NCORES)))

    att = np.empty((B, S), np.float32)
    ncov = np.empty((B, S), np.float32)
    for core in range(NCORES):
        r = res.results[core]
        # undo the per-128-block s flip: out partition p is s = 128j + 127 - p
        att[core * BPC:(core + 1) * BPC] = \
            r["att_out"][:, ::-1, :].transpose(0, 2, 1).reshape(BPC, S)
        ncov[core * BPC:(core + 1) * BPC] = \
            r["cov_out"][:, ::-1, :].transpose(0, 2, 1).reshape(BPC, S)
    return att, ncov
